# revision 1
# baseline (speedup 1.0000x reference)
"""Trainium2 Bass kernel for nn_MoDEChameleonMLP (MoDE Chameleon MLP).

Math (per token n):
  gate = x@Wg.T + delta_g(x); up = x@Wu.T + delta_u(x)
  inter = silu(gate)*up
  out  = inter@Wd.T + delta_d(inter)
where delta(v) = mask ? 2*(v@vA.T)@vB.T : 2*sum_e softmax(v@router.T)_e (v@A_e.T)@B_e.T

Implementation: token(B*S)-sharding across 8 cores (512 tokens/core, no
collectives). Each core:
  aux:    t = x@Acat.T (rank-40 LoRA bases + router logits), softmax routing +
          mask combine on device -> y = t * w  (transposed to [40,T], zero pad
          to 128 rows)
  phase1: gate/up = W-stationary matmuls producing [I-part, token] tiles,
          plus one extra K=128 matmul with the (pre-scaled) LoRA B matrix and
          y -> the full delta. silu(gate)*up -> inter resident in SBUF (bf16).
  phase2: same trick for the down projection, streaming Wd.
All matmuls bf16 with fp32 PSUM accumulation. Weights are host-side
transposed/pre-tiled so every device DMA is wide contiguous lines.
"""
import os
import sys

for p in ("/root/.axon_site/_ro/trn_rl_repo", "/opt/trn_rl_repo"):
    if os.path.isdir(p) and p not in sys.path:
        sys.path.append(p)

import numpy as np
import ml_dtypes

import concourse.bass as bass  # noqa: E402
import concourse.tile as tile  # noqa: E402
from concourse import bacc, mybir  # noqa: E402
from concourse.bass_utils import run_bass_kernel_spmd  # noqa: E402
from concourse.masks import make_identity  # noqa: E402

BF16 = ml_dtypes.bfloat16
BF = mybir.dt.bfloat16
F32 = mybir.dt.float32

NCORES = 8
T = 512          # tokens per core
TT = T // 128
SW = 256         # i-super width (2 x 128 psum tiles), divides 11008
E, R = 4, 8
SCALE = 2.0

_nc_cache = {}


def build_kernel(H, I):
    HB, IB = H // 128, I // 128
    NS, HS = I // SW, H // 512
    QH = HB // 4  # weight dma slices of 4 h-blocks

    nc = bacc.Bacc(None, target_bir_lowering=False)
    xt_d = nc.declare_dram_parameter("xt", [128, HB, T], BF, isOutput=False)
    mask_d = nc.declare_dram_parameter("maskf", [128, 2 * TT], F32, isOutput=False)
    acall_d = nc.declare_dram_parameter("acatall", [128, HB, 88], BF, isOutput=False)
    acd_d = nc.declare_dram_parameter("acatd", [128, IB, 44], BF, isOutput=False)
    wg_d = nc.declare_dram_parameter("wg", [NS, 128, HB, SW], BF, isOutput=False)
    wu_d = nc.declare_dram_parameter("wu", [NS, 128, HB, SW], BF, isOutput=False)
    bg_d = nc.declare_dram_parameter("bg", [NS, 128, SW], BF, isOutput=False)
    bu_d = nc.declare_dram_parameter("bu", [NS, 128, SW], BF, isOutput=False)
    wd_d = nc.declare_dram_parameter("wd", [HS, 128, IB, 512], BF, isOutput=False)
    bd_d = nc.declare_dram_parameter("bd", [HS, 128, 512], BF, isOutput=False)
    out_d = nc.declare_dram_parameter("out", [T, H], F32, isOutput=True)

    with tile.TileContext(nc) as tc:
        # The weight-stream pools (wstr for wg/wu, wstr2/bstr2 for wd/bd) are
        # opened up-front so their SBUF addresses are disjoint from every
        # scoped pool: their DMAs then have no address-release deps and
        # prefetch freely across phase boundaries (kills the PE stalls at
        # aux->phase1 and phase1->phase2 transitions).
        with tc.tile_pool(name="const", bufs=1) as constp, \
             tc.tile_pool(name="wstr", bufs=2 * QH + 4) as wstr, \
             tc.tile_pool(name="wstr2", bufs=8) as wstr2, \
             tc.tile_pool(name="bstr2", bufs=2) as bstr2:
            xt_sb = constp.tile([128, HB, T], BF)
            nc.sync.dma_start(xt_sb[:], xt_d[:])
            mask_sb = constp.tile([128, 2 * TT], F32)
            nc.sync.dma_start(mask_sb[:], mask_d[:])
            ident = constp.tile([128, 128], BF)
            make_identity(nc, ident)
            acall_sb = constp.tile([128, HB, 88], BF)
            nc.sync.dma_start(acall_sb[:], acall_d[:])
            acd_sb = constp.tile([128, IB, 44], BF)
            nc.sync.dma_start(acd_sb[:], acd_d[:])
            ygT = constp.tile([128, T], BF)
            yuT = constp.tile([128, T], BF)
            ydT = constp.tile([128, T], BF)
            for y in (ygT, yuT, ydT):
                nc.vector.memset(y[:], 0.0)
            inter_sb = constp.tile([128, IB, T], BF)

            def emit_route(auxps, auxtmp, ps, lo, vo, eo, t, yT):
                """softmax(ps[:,lo:lo+4]) routing + mask combine -> y, then
                transpose y[128,40] into yT[0:40, t*128:(t+1)*128]."""
                rmaxn = auxtmp.tile([128, 1], F32, tag="rmaxn", name=f"rx{t}")
                nc.vector.tensor_reduce(rmaxn, ps[:, lo:lo + 4],
                                        axis=mybir.AxisListType.X,
                                        op=mybir.AluOpType.max, negate=True)
                ee = auxtmp.tile([128, 4], F32, tag="ee", name=f"ee{t}")
                se = auxtmp.tile([128, 1], F32, tag="se", name=f"se{t}")
                nc.scalar.activation(ee, ps[:, lo:lo + 4],
                                     mybir.ActivationFunctionType.Exp,
                                     bias=rmaxn, accum_out=se)
                rec = auxtmp.tile([128, 1], F32, tag="rec", name=f"rc{t}")
                nc.vector.reciprocal(rec, se)
                r1m = auxtmp.tile([128, 1], F32, tag="r1m", name=f"rm{t}")
                nc.vector.tensor_tensor(r1m, rec, mask_sb[:, TT + t:TT + t + 1],
                                        mybir.AluOpType.mult)
                we = auxtmp.tile([128, 4], F32, tag="we", name=f"we{t}")
                nc.vector.tensor_scalar(we, ee, r1m, None, mybir.AluOpType.mult)
                yt = auxtmp.tile([128, 40], BF, tag="yt", name=f"yt{t}")
                nc.vector.tensor_scalar(yt[:, 0:8], ps[:, vo:vo + 8],
                                        mask_sb[:, t:t + 1], None,
                                        mybir.AluOpType.mult)
                for j in range(E):
                    nc.vector.tensor_scalar(yt[:, 8 + 8 * j:16 + 8 * j],
                                            ps[:, eo + 8 * j:eo + 8 * (j + 1)],
                                            we[:, j:j + 1], None,
                                            mybir.AluOpType.mult)
                tp = auxps.tile([128, 128], BF, tag="tp", name=f"tp{t}")
                nc.tensor.transpose(tp[:40, :], yt[:], ident)
                nc.vector.tensor_copy(yT[0:40, t * 128:(t + 1) * 128], tp[:40, :])

            # ---- aux pass for gate/up (rank-40 projections + router logits).
            # All 4 matmul chains are emitted before any routing so the PE
            # doesn't stall on the DVE routing chain at each transpose.
            with tc.tile_pool(name="auxps", bufs=4, space="PSUM") as auxps, \
                 tc.tile_pool(name="auxtmp", bufs=2) as auxtmp:
                pss = []
                for t in range(TT):
                    ps = auxps.tile([128, 512], F32, tag="aux", name=f"aux{t}")
                    for h in range(HB):
                        nc.tensor.matmul(ps[:, :88],
                                         xt_sb[:, h, t * 128:(t + 1) * 128],
                                         acall_sb[:, h, :],
                                         start=(h == 0), stop=(h == HB - 1))
                    pss.append(ps)
                for t in range(TT):
                    emit_route(auxps, auxtmp, pss[t], 80, 0, 8, t, ygT)
                    emit_route(auxps, auxtmp, pss[t], 84, 40, 48, t, yuT)

            # ---- phase 1: gate/up + silu*up -> inter (resident)
            with tc.tile_pool(name="bstr", bufs=3) as bstr, \
                 tc.tile_pool(name="etmp", bufs=3) as etmp, \
                 tc.tile_pool(name="mps", bufs=2, space="PSUM") as mps:
                NI2 = SW // 128
                for s in range(NS):
                    psgu = []
                    for proj, w_dram, b_dram in (("g", wg_d, bg_d), ("u", wu_d, bu_d)):
                        wt = []
                        for q in range(QH):
                            wq = wstr.tile([128, 4, SW], BF, tag="wt",
                                           name=f"w{proj}{s}_{q}")
                            nc.sync.dma_start(wq[:], w_dram[s, :, q * 4:(q + 1) * 4, :])
                            wt.append(wq)
                        bt = bstr.tile([128, SW], BF, tag="bt", name=f"b{proj}{s}")
                        nc.sync.dma_start(bt[:], b_dram[s])
                        yT = ygT if proj == "g" else yuT
                        pss = [mps.tile([128, 512], F32, tag=f"p{proj}{i2}",
                                        name=f"p{proj}{s}_{i2}") for i2 in range(NI2)]
                        for h in range(HB):
                            for i2 in range(NI2):
                                nc.tensor.matmul(pss[i2],
                                                 wt[h // 4][:, h % 4,
                                                            i2 * 128:(i2 + 1) * 128],
                                                 xt_sb[:, h, :],
                                                 start=(h == 0), stop=False)
                        for i2 in range(NI2):
                            nc.tensor.matmul(pss[i2], bt[:, i2 * 128:(i2 + 1) * 128],
                                             yT[:], start=False, stop=True)
                        psgu.append(pss)
                    for i2 in range(NI2):
                        i = s * NI2 + i2
                        st = etmp.tile([128, T], F32, tag="silu", name=f"si{s}_{i2}")
                        nc.scalar.activation(st[:], psgu[0][i2][:, :T],
                                             mybir.ActivationFunctionType.Silu)
                        nc.vector.tensor_tensor(inter_sb[:, i, :], st[:],
                                                psgu[1][i2][:, :T],
                                                mybir.AluOpType.mult)

            # ---- aux pass for down routing (over inter)
            with tc.tile_pool(name="auxps2", bufs=4, space="PSUM") as auxps2, \
                 tc.tile_pool(name="auxtmp2", bufs=2) as auxtmp2:
                pss = []
                for t in range(TT):
                    ps = auxps2.tile([128, 512], F32, tag="auxd", name=f"auxd{t}")
                    for i in range(IB):
                        nc.tensor.matmul(ps[:, :44],
                                         inter_sb[:, i, t * 128:(t + 1) * 128],
                                         acd_sb[:, i, :],
                                         start=(i == 0), stop=(i == IB - 1))
                    pss.append(ps)
                for t in range(TT):
                    emit_route(auxps2, auxtmp2, pss[t], 0, 4, 12, t, ydT)

            # ---- phase 2: down projection
            with tc.tile_pool(name="ost", bufs=3) as ost, \
                 tc.tile_pool(name="ops", bufs=2, space="PSUM") as ops:
                for hs in range(HS):
                    bdt = bstr2.tile([128, 512], BF, tag="bd2", name=f"bd{hs}")
                    nc.sync.dma_start(bdt[:], bd_d[hs])
                    pso = [ops.tile([128, 512], F32, tag=f"po{t}",
                                    name=f"po{hs}_{t}") for t in range(TT)]
                    for i in range(IB):
                        wdt = wstr2.tile([128, 512], BF, tag="wd2",
                                         name=f"wd{hs}_{i}")
                        nc.sync.dma_start(wdt[:], wd_d[hs, :, i, :])
                        for t in range(TT):
                            nc.tensor.matmul(pso[t],
                                             inter_sb[:, i, t * 128:(t + 1) * 128],
                                             wdt[:], start=(i == 0), stop=False)
                    for t in range(TT):
                        nc.tensor.matmul(pso[t], ydT[:, t * 128:(t + 1) * 128],
                                         bdt[:], start=False, stop=True)
                        osb = ost.tile([128, 512], F32, tag="os", name=f"os{hs}_{t}")
                        nc.vector.tensor_copy(osb[:], pso[t][:])
                        nc.sync.dma_start(
                            out_d[t * 128:(t + 1) * 128, hs * 512:(hs + 1) * 512],
                            osb[:])
    nc.finalize()
    return nc


def get_nc(H, I):
    key = (H, I)
    if key not in _nc_cache:
        _nc_cache[key] = build_kernel(H, I)
    return _nc_cache[key]


def _prep_weights(Wg, Wu, Wd, va_gate_A, va_gate_B, va_up_A, va_up_B,
                  va_down_A, va_down_B, router_gate, tm_gate_A, tm_gate_B,
                  router_up, tm_up_A, tm_up_B, router_down, tm_down_A, tm_down_B):
    I, H = Wg.shape
    HB, IB = H // 128, I // 128
    NS, HS = I // SW, H // 512

    def tile_w_ih(W):  # [I,H] -> [NS,128,HB,SW]; w[s,p,h,c]=W[s*SW+c, h*128+p]
        return np.ascontiguousarray(
            W.reshape(NS, SW, HB, 128).transpose(0, 3, 2, 1)).astype(BF16)

    def tile_bcat(vB, tB, rows):  # -> [nblk,128,blk]; padded 2*[vB|tB_e].T
        out_dim = vB.shape[0]
        Bcat = np.concatenate([vB] + [tB[e] for e in range(E)], axis=1)  # [out,40]
        Bp = np.zeros((128, out_dim), np.float32)
        Bp[:40, :] = SCALE * Bcat.T
        blk = out_dim // rows
        return np.ascontiguousarray(
            Bp.reshape(128, rows, blk).transpose(1, 0, 2)).astype(BF16)

    A_all = np.concatenate([va_gate_A, tm_gate_A.reshape(E * R, H),
                            va_up_A, tm_up_A.reshape(E * R, H),
                            router_gate, router_up], axis=0)  # [88,H]
    acatall = np.ascontiguousarray(
        A_all.T.reshape(HB, 128, 88).transpose(1, 0, 2)).astype(BF16)
    A_d = np.concatenate([router_down, va_down_A,
                          tm_down_A.reshape(E * R, I)], axis=0)  # [44,I]
    acatd = np.ascontiguousarray(
        A_d.T.reshape(IB, 128, 44).transpose(1, 0, 2)).astype(BF16)

    wd = np.ascontiguousarray(
        Wd.reshape(HS, 512, IB, 128).transpose(0, 3, 2, 1)).astype(BF16)

    return {
        "acatall": acatall,
        "acatd": acatd,
        "wg": tile_w_ih(Wg),
        "wu": tile_w_ih(Wu),
        "bg": tile_bcat(va_gate_B, tm_gate_B, NS),
        "bu": tile_bcat(va_up_B, tm_up_B, NS),
        "wd": wd,
        "bd": tile_bcat(va_down_B, tm_down_B, HS),
    }


def _prep_core_inputs(x, image_mask, weights, n_cores):
    Bb, S, H = x.shape
    HB = H // 128
    xf = np.asarray(x, np.float32).reshape(-1, H)
    m = np.asarray(image_mask).reshape(-1).astype(np.float32)
    in_maps = []
    for c in range(n_cores):
        sh = xf[c * T:(c + 1) * T]                      # [T,H]
        xt = np.ascontiguousarray(
            sh.T.reshape(HB, 128, T).transpose(1, 0, 2)).astype(BF16)
        mc = m[c * T:(c + 1) * T].reshape(TT, 128).T    # [128,TT]
        maskf = np.ascontiguousarray(
            np.concatenate([mc, 1.0 - mc], axis=1)).astype(np.float32)
        in_maps.append({"xt": xt, "maskf": maskf, **weights})
    return in_maps


def run(x, image_mask, weights_raw, trace=False):
    Bb, S, H = x.shape
    I = weights_raw["Wg"].shape[0]
    nc = get_nc(H, I)
    weights = _prep_weights(**weights_raw)
    in_maps = _prep_core_inputs(x, image_mask, weights, NCORES)
    res = run_bass_kernel_spmd(nc, in_maps, list(range(NCORES)), trace=trace)
    out = np.concatenate([r["out"] for r in res.results], axis=0)
    return out.reshape(Bb, S, H).astype(np.float32), res


def kernel(x, image_mask, Wg, Wu, Wd,
           va_gate_A, va_gate_B, va_up_A, va_up_B, va_down_A, va_down_B,
           router_gate, tm_gate_A, tm_gate_B,
           router_up, tm_up_A, tm_up_B,
           router_down, tm_down_A, tm_down_B):
    weights_raw = dict(
        Wg=np.asarray(Wg, np.float32), Wu=np.asarray(Wu, np.float32),
        Wd=np.asarray(Wd, np.float32),
        va_gate_A=np.asarray(va_gate_A), va_gate_B=np.asarray(va_gate_B),
        va_up_A=np.asarray(va_up_A), va_up_B=np.asarray(va_up_B),
        va_down_A=np.asarray(va_down_A), va_down_B=np.asarray(va_down_B),
        router_gate=np.asarray(router_gate), tm_gate_A=np.asarray(tm_gate_A),
        tm_gate_B=np.asarray(tm_gate_B),
        router_up=np.asarray(router_up), tm_up_A=np.asarray(tm_up_A),
        tm_up_B=np.asarray(tm_up_B),
        router_down=np.asarray(router_down), tm_down_A=np.asarray(tm_down_A),
        tm_down_B=np.asarray(tm_down_B),
    )
    out, _ = run(np.asarray(x), np.asarray(image_mask), weights_raw, trace=False)
    return out

